# revision 1
# baseline (speedup 1.0000x reference)
"""Multi-head attention (N=4, T=2048, D=512, H=8, dh=64) on 8 TRN2 NeuronCores.

Sharding: batch N (4) x head-group (2 groups of 4 heads) -> 8 cores.
Each core computes, for its (batch n, head-group g):
  q = query[n] @ Wq[:, 256g:256g+256]   (as qT, [256, 2048])
  k = key[n]   @ Wk[:, ...]             (as kT)
  v = key[n]   @ Wv[:, ...]             (as V tiles [t, dh] with ones column)
  per head h' in 0..3, per q-block of 512:
    ST[k, q] = K-tile matmuls (contraction dh=64, bf16)
    P = exp(ST / sqrt(512))  (ScalarE, multi-bank PSUM read)
    OT[65, 512] += [V | 1]^T @ P  (row 64 = softmax denominators)
    out = OT[0:64] * broadcast(1 / OT[64])
Host reassembles out[n, :, 256g:256g+256] = oT.T.

The attention loop is software-pipelined: pair p's score/exp phase is
interleaved with pair p-1's O-accumulation so the in-order TensorE queue
never parks O matmuls behind unfinished exps.
"""

import math

import ml_dtypes
import numpy as np

import concourse.bass as bass
import concourse.mybir as mybir
import concourse.tile as tile
from concourse import bacc
from concourse.bass_utils import run_bass_kernel_spmd

F32 = mybir.dt.float32
BF16 = mybir.dt.bfloat16
EXP = mybir.ActivationFunctionType.Exp

N, T, D = 4, 2048, 512
HPC, DH = 4, 64          # heads per core, head dim
GC = HPC * DH            # head-group columns (256)
SCALE = 1.0 / math.sqrt(D)
QB = 512                 # q block
NQB = T // QB            # 4
NKT = T // 128           # 16 k tiles
KS = D // 128            # 4 contraction slices for projections

# exp-group pattern per (head, qblock): (pool_key, n_ktiles). Pools A (4 banks)
# and B (2 banks) alternate so TensorE score matmuls overlap ScalarE exp.
GROUPS = (("A", 2), ("B", 2), ("A", 4), ("B", 2), ("A", 4), ("B", 2))


def build():
    nc = bacc.Bacc("TRN2", target_bir_lowering=False, debug=False, num_devices=8)
    qT_in = nc.declare_dram_parameter("qT", [D, T], BF16, isOutput=False)
    kT_in = nc.declare_dram_parameter("kT", [D, T], BF16, isOutput=False)
    wq_in = nc.declare_dram_parameter("wq", [D, GC], BF16, isOutput=False)
    wk_in = nc.declare_dram_parameter("wk", [D, GC], BF16, isOutput=False)
    wv_in = nc.declare_dram_parameter("wv", [D, GC], BF16, isOutput=False)
    oT_out = nc.declare_dram_parameter("oT", [GC, T], F32, isOutput=True)

    with tile.TileContext(nc) as tc:
        with (
            tc.tile_pool(name="stage", bufs=8) as stage,
            tc.tile_pool(name="const", bufs=1) as const,
            tc.tile_pool(name="act", bufs=1) as actp,
            tc.tile_pool(name="pt", bufs=3) as ptp,
            tc.tile_pool(name="small", bufs=4) as small,
            tc.tile_pool(name="psA", bufs=1, space="PSUM") as psA,
            tc.tile_pool(name="psB", bufs=1, space="PSUM") as psB,
            tc.tile_pool(name="psC", bufs=2, space="PSUM") as psC,
        ):
            # ---- weights ----
            ws = {}
            for nm, src in (("wq", wq_in), ("wk", wk_in), ("wv", wv_in)):
                w = const.tile([128, KS, GC], BF16, tag=nm)
                nc.sync.dma_start(w[:], src.rearrange("(s p) c -> p s c", p=128))
                ws[nm] = w

            # ---- key^T staging ----
            kin = []
            for s in range(KS):
                t_ = stage.tile([128, T], BF16, tag="qkin", name=f"kin{s}")
                kin.append(t_)
            for tb in range(NQB):
                for s in range(KS):
                    nc.sync.dma_start(
                        kin[s][:, tb * QB : (tb + 1) * QB],
                        kT_in[s * 128 : (s + 1) * 128, tb * QB : (tb + 1) * QB],
                    )

            # ---- query^T staging (own slots; DMAs overlap k/v projection) ----
            qin = []
            for s in range(KS):
                t_ = stage.tile([128, T], BF16, tag="qkin", name=f"qin{s}")
                qin.append(t_)
            for tb in range(NQB):
                for s in range(KS):
                    nc.sync.dma_start(
                        qin[s][:, tb * QB : (tb + 1) * QB],
                        qT_in[s * 128 : (s + 1) * 128, tb * QB : (tb + 1) * QB],
                    )

            # ---- kT projection: kT_att[dt][p, t] = (key @ Wk)^T ----
            kT_att = [
                actp.tile([128, T], BF16, tag=f"ka{d}", name=f"ka{d}")
                for d in range(2)
            ]
            qT_att = [
                actp.tile([128, T], BF16, tag=f"qa{d}", name=f"qa{d}")
                for d in range(2)
            ]
            for dt2 in range(2):
                for tb in range(NQB):
                    ps = psC.tile([128, QB], F32, tag="C")
                    for s in range(KS):
                        nc.tensor.matmul(
                            ps[:],
                            ws["wk"][:, s, dt2 * 128 : (dt2 + 1) * 128],
                            kin[s][:, tb * QB : (tb + 1) * QB],
                            start=(s == 0),
                            stop=(s == KS - 1),
                        )
                    nc.vector.tensor_copy(
                        kT_att[dt2][:, tb * QB : (tb + 1) * QB], ps[:]
                    )

            # ---- V projection into [128, kt, head, 65] with ones column ----
            vp = const.tile([128, NKT, HPC, DH + 1], BF16, tag="vp")
            ones_f32 = const.tile([128, NKT * HPC], F32, tag="ones")
            nc.gpsimd.memset(ones_f32[:], 1.0)
            nc.vector.tensor_copy(
                vp[:, :, :, DH : DH + 1],
                ones_f32[:].rearrange("p (a b) -> p a b", b=HPC).unsqueeze(3),
            )
            for tt in range(NKT):
                ps = psC.tile([128, QB], F32, tag="C")
                for s in range(KS):
                    nc.tensor.matmul(
                        ps[:, 0:GC],
                        kin[s][:, tt * 128 : (tt + 1) * 128],
                        ws["wv"][:, s, :],
                        start=(s == 0),
                        stop=(s == KS - 1),
                    )
                nc.vector.tensor_copy(
                    vp[:, tt, :, 0:DH],
                    ps[:, 0:GC].rearrange("p (h d) -> p h d", d=DH),
                )

            # ---- attention, software-pipelined ----
            def emit_qproj(qb):
                for dt2 in range(2):
                    ps = psC.tile([128, QB], F32, tag="C", name="qproj_ps")
                    for s in range(KS):
                        nc.tensor.matmul(
                            ps[:],
                            ws["wq"][:, s, dt2 * 128 : (dt2 + 1) * 128],
                            qin[s][:, qb * QB : (qb + 1) * QB],
                            start=(s == 0),
                            stop=(s == KS - 1),
                        )
                    nc.vector.tensor_copy(
                        qT_att[dt2][:, qb * QB : (qb + 1) * QB], ps[:]
                    )

            def emit_s_group(qb, hp, pt, gi):
                pool_key, nkt = GROUPS[gi]
                kt0 = sum(n for _, n in GROUPS[:gi])
                tile2, base = hp // 2, DH * (hp % 2)
                q_src = qT_att[tile2][base : base + DH, qb * QB : (qb + 1) * QB]
                pool = psA if pool_key == "A" else psB
                width = 2048 if pool_key == "A" else 1024
                ps = pool.tile([128, width], F32, tag=pool_key, name="s_ps")
                for l in range(nkt):
                    kt = kt0 + l
                    nc.tensor.matmul(
                        ps[:, l * QB : (l + 1) * QB],
                        kT_att[tile2][base : base + DH, kt * 128 : (kt + 1) * 128],
                        q_src,
                        start=True,
                        stop=True,
                    )
                nc.scalar.activation(
                    pt[:, kt0 * QB : (kt0 + nkt) * QB],
                    ps[:, : nkt * QB],
                    EXP,
                    scale=SCALE,
                )

            def emit_o_chunk(prev, kt_lo, kt_hi):
                qb, hp, pt, po = prev
                for kt in range(kt_lo, kt_hi):
                    nc.tensor.matmul(
                        po[0 : DH + 1],
                        vp[:, kt, hp, :],
                        pt[:, kt * QB : (kt + 1) * QB],
                        start=(kt == 0),
                        stop=(kt == NKT - 1),
                    )

            def emit_norm(prev):
                qb, hp, pt, po = prev
                sums = small.tile([1, QB], F32, tag="sums", name="sums")
                nc.vector.tensor_copy(sums[:], po[DH : DH + 1, :])
                rec = small.tile([1, QB], F32, tag="rec", name="rec")
                nc.vector.reciprocal_approx_fast(rec[:], sums[:])
                bc = small.tile([DH, QB], F32, tag="bc", name="bc")
                nc.gpsimd.partition_broadcast(bc[:], rec[:])
                ot = small.tile([DH, QB], F32, tag="ot", name="ot")
                nc.vector.tensor_mul(ot[:], po[0:DH, :], bc[:])
                nc.gpsimd.dma_start(
                    oT_out[hp * DH : (hp + 1) * DH, qb * QB : (qb + 1) * QB],
                    ot[:],
                )

            pairs = [(qb, hp) for qb in range(NQB) for hp in range(HPC)]
            prev = None
            for qb, hp in pairs:
                if hp == 0:
                    emit_qproj(qb)
                pt = ptp.tile([128, NKT * QB], BF16, tag="pt", name="pt")
                if prev is not None:
                    po_prev = psC.tile([128, QB], F32, tag="C", name="po")
                    prev = (*prev, po_prev)
                emit_s_group(qb, hp, pt, 0)
                emit_s_group(qb, hp, pt, 1)
                if prev is not None:
                    emit_o_chunk(prev, 0, 8)
                emit_s_group(qb, hp, pt, 2)
                emit_s_group(qb, hp, pt, 3)
                if prev is not None:
                    emit_o_chunk(prev, 8, NKT)
                emit_s_group(qb, hp, pt, 4)
                emit_s_group(qb, hp, pt, 5)
                if prev is not None:
                    emit_norm(prev)
                prev = (qb, hp, pt)
            po_prev = psC.tile([128, QB], F32, tag="C", name="po")
            prev = (*prev, po_prev)
            emit_o_chunk(prev, 0, NKT)
            emit_norm(prev)

    nc.compile()
    return nc


_NC = None


def _get_nc():
    global _NC
    if _NC is None:
        _NC = build()
    return _NC


def run(query, key, W_query, W_key, W_value, trace=False):
    nc = _get_nc()
    query = np.asarray(query, dtype=np.float32)
    key = np.asarray(key, dtype=np.float32)
    W_query = np.asarray(W_query, dtype=np.float32)
    W_key = np.asarray(W_key, dtype=np.float32)
    W_value = np.asarray(W_value, dtype=np.float32)

    in_maps = []
    for c in range(8):
        n, g = c // 2, c % 2
        cols = slice(g * GC, (g + 1) * GC)
        in_maps.append(
            {
                "qT": np.ascontiguousarray(query[n].T.astype(ml_dtypes.bfloat16)),
                "kT": np.ascontiguousarray(key[n].T.astype(ml_dtypes.bfloat16)),
                "wq": np.ascontiguousarray(W_query[:, cols].astype(ml_dtypes.bfloat16)),
                "wk": np.ascontiguousarray(W_key[:, cols].astype(ml_dtypes.bfloat16)),
                "wv": np.ascontiguousarray(W_value[:, cols].astype(ml_dtypes.bfloat16)),
            }
        )
    res = run_bass_kernel_spmd(nc, in_maps, core_ids=list(range(8)), trace=trace)
    out = np.empty((N, T, D), dtype=np.float32)
    for c in range(8):
        n, g = c // 2, c % 2
        out[n, :, g * GC : (g + 1) * GC] = res.results[c]["oT"].T
    return out, res


def kernel(query, key, W_query, W_key, W_value):
    out, _ = run(query, key, W_query, W_key, W_value, trace=False)
    return out

